# revision 7
# baseline (speedup 1.0000x reference)
"""Distributed 2-layer GCN (PyG GCNConv semantics) on 8 Trainium2 NeuronCores.

Strategy (per sharding hint): nodes are sharded across the 8 cores
(12500 nodes each); edges are bucketed by destination core/tile via 1D
graph partitioning on the host. Three SPMD launches:

  1. transform1:  ht1 = dinv * (x @ W1)          (each core: its node shard)
     -- host concatenates the 8 shards into the full ht1 table --
  2. agg1+xform2: per dst tile: one-hot matmul segmented sum over
     dma_gather'ed ht1[src] rows, + self loop + bias + relu, then
     ht2 = dinv * (h1 @ W2)
     -- host concatenates ht2 shards --
  3. agg2+logsoftmax: same aggregation over ht2, + bias, log_softmax.

The aggregation uses sorted edge buckets per (dst-tile, src-segment)
padded to 128-edge blocks; each block is reduced with a matmul whose
stationary operand is a one-hot selection matrix built on the vector
engine (iota == dst_local), accumulating into PSUM per dst tile.
Gathers use gpsimd.dma_gather (int16 indices -> 4 source segments of
25000 rows each).
"""

import os
import sys
import time
import types

for _p in ("/opt/trn_rl_repo", "/root/.axon_site/_ro/trn_rl_repo", "/root/.axon_site"):
    if os.path.isdir(_p) and _p not in sys.path:
        sys.path.insert(0, _p)

import numpy as np

from concourse import bass, bacc, tile
from concourse.bass_utils import run_bass_kernel_spmd

mybir = bass.mybir
DT = bass.mybir.dt
ALU = mybir.AluOpType
ACTF = mybir.ActivationFunctionType

# ----------------------------------------------------------------------------
# Configuration
# ----------------------------------------------------------------------------

class Cfg:
    def __init__(self, N=100000, E=1600000, F0=256, F1=128, F2=64,
                 NCORES=8, SEG=4, TG=4):
        self.N = N
        self.E = E
        self.F0 = F0
        self.F1 = F1
        self.F2 = F2
        self.NCORES = NCORES
        self.NPC = N // NCORES            # nodes per core
        self.NT = -(-self.NPC // 128)     # dst tiles per core
        self.LAST_ROWS = self.NPC - (self.NT - 1) * 128
        self.SEG = SEG                    # src segments (int16 gather indices)
        assert N % SEG == 0
        self.SEGSZ = N // SEG
        assert self.SEGSZ <= 32767
        self.TG = TG                      # dst tiles per gather group
        self.NG = -(-self.NT // TG)
        self.groups = [list(range(g * TG, min((g + 1) * TG, self.NT)))
                       for g in range(self.NG)]


class Meta:
    """Edge partitioning metadata; identical across cores (static program)."""
    pass


def preprocess(cfg, edge_index):
    """1D graph partitioning of the edge list. Pure integer index work."""
    src = np.asarray(edge_index[0], dtype=np.int64)
    dst = np.asarray(edge_index[1], dtype=np.int64)

    cnt = np.bincount(dst, minlength=cfg.N).astype(np.int64)

    core = dst // cfg.NPC
    within = dst % cfg.NPC
    tile_id = within // 128
    dloc = within % 128
    seg = src // cfg.SEGSZ
    sloc = src % cfg.SEGSZ

    # bucket id (core, tile, seg); sort edges by (bucket, sloc) for locality
    bucket = (core * cfg.NT + tile_id) * cfg.SEG + seg
    order = np.argsort(bucket * np.int64(cfg.SEGSZ) + sloc, kind="stable")
    b_sorted = bucket[order]
    sloc_sorted = sloc[order].astype(np.int16)
    dloc_sorted = dloc[order].astype(np.float32)

    nbuckets = cfg.NCORES * cfg.NT * cfg.SEG
    bc = np.bincount(bucket, minlength=nbuckets).reshape(cfg.NCORES, cfg.NT, cfg.SEG)
    bstart = np.zeros(nbuckets + 1, np.int64)
    np.cumsum(bc.reshape(-1), out=bstart[1:])

    # static per-(tile, seg) block counts = max over cores, ceil to blocks
    nblk = -(-bc.max(axis=0) // 128)          # [NT, SEG]
    nblk = np.maximum(nblk, 0)

    m = Meta()
    m.nblk = nblk
    # slot layout: group -> seg -> tiles in group -> blocks
    m.ns = np.zeros((cfg.NG, cfg.SEG), np.int64)       # slots per (group, seg)
    m.goff = np.zeros((cfg.NG, cfg.SEG), np.int64)     # global slot offset
    m.lco = {}                                          # (g, s, t) -> local block col
    off = 0
    for g, tiles in enumerate(cfg.groups):
        for s in range(cfg.SEG):
            m.goff[g, s] = off
            lc = 0
            for t in tiles:
                m.lco[(g, s, t)] = lc
                lc += int(nblk[t, s])
            m.ns[g, s] = lc * 128
            off += lc * 128
    m.tot = off
    assert m.tot % 128 == 0

    # per-core slot arrays
    idx_all = np.zeros((cfg.NCORES, m.tot), np.int16)
    dl_all = np.full((cfg.NCORES, m.tot), -1.0, np.float32)
    for c in range(cfg.NCORES):
        for g, tiles in enumerate(cfg.groups):
            for s in range(cfg.SEG):
                for t in tiles:
                    b = (c * cfg.NT + t) * cfg.SEG + s
                    k = int(bc[c, t, s])
                    if k == 0:
                        continue
                    e0 = int(bstart[b])
                    o = int(m.goff[g, s]) + m.lco[(g, s, t)] * 128
                    idx_all[c, o:o + k] = sloc_sorted[e0:e0 + k]
                    dl_all[c, o:o + k] = dloc_sorted[e0:e0 + k]

    # device layouts
    # idx: slot j -> [j % 16, j // 16], replicated over the 8 stripes of 16
    idx_dev = np.ascontiguousarray(
        np.tile(idx_all.reshape(cfg.NCORES, m.tot // 16, 16).transpose(0, 2, 1),
                (1, 8, 1)))
    # dstloc: slot j -> [j % 128, j // 128]
    dl_dev = np.ascontiguousarray(
        dl_all.reshape(cfg.NCORES, m.tot // 128, 128).transpose(0, 2, 1))

    # degree counts per core as f32 [128, NT] (node t*128+p <-> [p, t])
    pad = cfg.NT * 128 - cfg.NPC
    cnt_dev = np.zeros((cfg.NCORES, 128, cfg.NT), np.float32)
    for c in range(cfg.NCORES):
        cc = cnt[c * cfg.NPC:(c + 1) * cfg.NPC]
        cc = np.concatenate([cc, np.zeros(pad, np.int64)])
        cnt_dev[c] = cc.reshape(cfg.NT, 128).T.astype(np.float32)

    m.idx_dev = idx_dev
    m.dl_dev = dl_dev
    m.cnt_dev = cnt_dev
    return m


# ----------------------------------------------------------------------------
# Program builders
# ----------------------------------------------------------------------------

def _dinv_tiles(nc, pool, cnt_in, cfg):
    """dinv = 1/sqrt(cnt + 1) as an SBUF [128, NT] f32 tile."""
    cnt_sb = pool.tile([128, cfg.NT], DT.float32, tag="cnt")
    nc.sync.dma_start(out=cnt_sb[:], in_=cnt_in[:])
    deg = pool.tile([128, cfg.NT], DT.float32, tag="deg")
    nc.vector.tensor_scalar_add(deg[:], cnt_sb[:], 1.0)
    sq = pool.tile([128, cfg.NT], DT.float32, tag="sq")
    nc.scalar.sqrt(sq[:], deg[:])
    dinv = pool.tile([128, cfg.NT], DT.float32, tag="dinv")
    nc.vector.reciprocal(dinv[:], sq[:])
    return dinv


def build_transform1(cfg):
    """ht1 = dinv * (x @ W1) for the local node shard."""
    nc = bacc.Bacc(None, target_bir_lowering=False)
    x_in = nc.declare_dram_parameter("x", [cfg.NPC, cfg.F0], DT.float32, isOutput=False)
    w1_in = nc.declare_dram_parameter("w1", [cfg.F0, cfg.F1], DT.float32, isOutput=False)
    cnt_in = nc.declare_dram_parameter("cnt", [128, cfg.NT], DT.float32, isOutput=False)
    id_in = nc.declare_dram_parameter("ident", [128, 128], DT.float32, isOutput=False)
    ht_out = nc.declare_dram_parameter("ht1", [cfg.NPC, cfg.F1], DT.float32, isOutput=True)

    KB = cfg.F0 // 128
    with tile.TileContext(nc) as tc:
        with tc.tile_pool(name="const", bufs=1) as cpool, \
             tc.tile_pool(name="work", bufs=3) as wpool, \
             tc.tile_pool(name="psum", bufs=2, space="PSUM") as ppool:
            ident = cpool.tile([128, 128], DT.float32, tag="ident")
            nc.sync.dma_start(out=ident[:], in_=id_in[:])
            dinv = _dinv_tiles(nc, cpool, cnt_in, cfg)
            w1sb = []
            for kb in range(KB):
                w = cpool.tile([128, cfg.F1], DT.float32, tag=f"w1_{kb}")
                nc.sync.dma_start(out=w[:], in_=w1_in[kb * 128:(kb + 1) * 128, :])
                w1sb.append(w)

            for t in range(cfg.NT):
                rows = cfg.LAST_ROWS if t == cfg.NT - 1 else 128
                xt = wpool.tile([128, cfg.F0], DT.float32, tag="xt")
                nc.sync.dma_start(out=xt[:rows, :],
                                  in_=x_in[t * 128:t * 128 + rows, :])
                xTs = []
                for kb in range(KB):
                    xTp = ppool.tile([128, 128], DT.float32, tag="xTp")
                    nc.tensor.transpose(xTp[:, :rows],
                                        xt[:rows, kb * 128:(kb + 1) * 128],
                                        ident[:rows, :rows])
                    xTc = wpool.tile([128, 128], DT.float32, tag=f"xT_{kb}")
                    nc.vector.tensor_copy(xTc[:, :rows], xTp[:, :rows])
                    xTs.append(xTc)
                hp = ppool.tile([128, cfg.F1], DT.float32, tag="hp")
                for kb in range(KB):
                    nc.tensor.matmul(hp[:rows, :], xTs[kb][:, :rows], w1sb[kb][:],
                                     start=(kb == 0), stop=(kb == KB - 1))
                hts = wpool.tile([128, cfg.F1], DT.float32, tag="hts")
                nc.vector.tensor_scalar_mul(hts[:rows, :], hp[:rows, :],
                                            dinv[:rows, t:t + 1])
                nc.sync.dma_start(out=ht_out[t * 128:t * 128 + rows, :],
                                  in_=hts[:rows, :])
    nc.compile()
    return nc


def build_agg(cfg, meta, layer):
    """layer 1: aggregate ht1 -> h1 -> ht2 = dinv*(h1 @ W2). Output "ht2".
       layer 2: aggregate ht2 -> +b2 -> log_softmax. Output "out"."""
    FIN = cfg.F1 if layer == 1 else cfg.F2
    nc = bacc.Bacc(None, target_bir_lowering=False)
    tab_in = [nc.declare_dram_parameter(f"tab{si}", [cfg.SEGSZ, FIN], DT.float32,
                                        isOutput=False) for si in range(cfg.SEG)]
    own_in = nc.declare_dram_parameter("own", [cfg.NPC, FIN], DT.float32, isOutput=False)
    cnt_in = nc.declare_dram_parameter("cnt", [128, cfg.NT], DT.float32, isOutput=False)
    idx_in = nc.declare_dram_parameter("idx", [128, meta.tot // 16], DT.int16, isOutput=False)
    dl_in = nc.declare_dram_parameter("dl", [128, meta.tot // 128], DT.float32, isOutput=False)
    iota_in = nc.declare_dram_parameter("iota", [128, 128], DT.float32, isOutput=False)
    b_in = nc.declare_dram_parameter("bvec", [128, FIN], DT.float32, isOutput=False)
    if layer == 1:
        id_in = nc.declare_dram_parameter("ident", [128, 128], DT.float32, isOutput=False)
        w2_in = nc.declare_dram_parameter("w2", [cfg.F1, cfg.F2], DT.float32, isOutput=False)
        out_t = nc.declare_dram_parameter("ht2", [cfg.NPC, cfg.F2], DT.float32, isOutput=True)
    else:
        out_t = nc.declare_dram_parameter("out", [cfg.NPC, cfg.F2], DT.float32, isOutput=True)

    nblk = meta.nblk
    with tile.TileContext(nc) as tc:
        with tile_pools(tc, cfg) as (cpool, idxpool, gpools, spool, wpool, ppool):
            iota = cpool.tile([128, 128], DT.float32, tag="iota")
            nc.sync.dma_start(out=iota[:], in_=iota_in[:])
            bvec = cpool.tile([128, FIN], DT.float32, tag="bvec")
            nc.sync.dma_start(out=bvec[:], in_=b_in[:])
            dinv = _dinv_tiles(nc, cpool, cnt_in, cfg)
            if layer == 1:
                ident = cpool.tile([128, 128], DT.float32, tag="ident")
                nc.sync.dma_start(out=ident[:], in_=id_in[:])
                w2sb = cpool.tile([128, cfg.F2], DT.float32, tag="w2")
                nc.sync.dma_start(out=w2sb[:], in_=w2_in[:])

            idx_sb = idxpool.tile([128, meta.tot // 16], DT.int16, tag="idx")
            nc.sync.dma_start(out=idx_sb[:], in_=idx_in[:])
            dl_sb = idxpool.tile([128, meta.tot // 128], DT.float32, tag="dl")
            nc.sync.dma_start(out=dl_sb[:], in_=dl_in[:])

            max_groups = int(os.environ.get("GCN_MAX_GROUPS", "0")) or len(cfg.groups)
            stage = os.environ.get("GCN_STAGE", "full")
            for g, tiles in enumerate(cfg.groups[:max_groups]):
                Gt = {}
                for s in range(cfg.SEG):
                    ns = int(meta.ns[g, s])
                    if ns == 0:
                        continue
                    Gs = gpools[s].tile([128, ns // 128, FIN], DT.float32,
                                        tag=f"G{s}")
                    o16 = int(meta.goff[g, s]) // 16
                    nc.gpsimd.dma_gather(
                        out_ap=Gs[:],
                        in_ap=tab_in[s][:, :],
                        idxs_ap=idx_sb[:, o16:o16 + ns // 16],
                        num_idxs=ns,
                        num_idxs_reg=ns,
                        elem_size=FIN,
                        single_packet=False,
                    )
                    Gt[s] = Gs
                if stage == "gather":
                    continue

                for t in tiles:
                    rows = cfg.LAST_ROWS if t == cfg.NT - 1 else 128
                    nbt = int(nblk[t].sum())
                    acc = ppool.tile([128, FIN], DT.float32, tag="acc")
                    bi = 0
                    for s in range(cfg.SEG):
                        lco = meta.lco[(g, s, t)]
                        for j in range(int(nblk[t, s])):
                            gcol = int(meta.goff[g, s]) // 128 + lco + j
                            S = spool.tile([128, 128], DT.float32, tag="S")
                            nc.vector.tensor_scalar(
                                S[:], iota[:], dl_sb[:, gcol:gcol + 1], None,
                                op0=ALU.is_equal)
                            nc.tensor.matmul(acc[:], S[:], Gt[s][:, lco + j, :],
                                             start=(bi == 0), stop=(bi == nbt - 1))
                            bi += 1
                    assert bi == nbt and nbt > 0
                    if stage == "agg":
                        zz = wpool.tile([128, FIN], DT.float32, tag="zz")
                        nc.vector.tensor_copy(zz[:rows, :], acc[:rows, :])
                        nc.sync.dma_start(out=out_t[t * 128:t * 128 + rows, :cfg.F2]
                                          if layer == 1 else
                                          out_t[t * 128:t * 128 + rows, :],
                                          in_=zz[:rows, :cfg.F2])
                        continue

                    ownt = wpool.tile([128, FIN], DT.float32, tag="own")
                    nc.sync.dma_start(out=ownt[:rows, :],
                                      in_=own_in[t * 128:t * 128 + rows, :])
                    z = wpool.tile([128, FIN], DT.float32, tag="z")
                    nc.vector.tensor_tensor(z[:rows, :], acc[:rows, :],
                                            ownt[:rows, :], op=ALU.add)
                    nc.vector.tensor_scalar_mul(z[:rows, :], z[:rows, :],
                                                dinv[:rows, t:t + 1])
                    nc.vector.tensor_tensor(z[:rows, :], z[:rows, :],
                                            bvec[:rows, :], op=ALU.add)
                    if layer == 1:
                        h1 = wpool.tile([128, cfg.F1], DT.float32, tag="h1")
                        nc.scalar.activation(h1[:rows, :], z[:rows, :], ACTF.Relu)
                        hTp = ppool.tile([128, 128], DT.float32, tag="hTp")
                        nc.tensor.transpose(hTp[:, :rows], h1[:rows, :],
                                            ident[:rows, :rows])
                        hT = wpool.tile([128, 128], DT.float32, tag="hT")
                        nc.vector.tensor_copy(hT[:, :rows], hTp[:, :rows])
                        t2p = ppool.tile([128, cfg.F2], DT.float32, tag="t2p")
                        nc.tensor.matmul(t2p[:rows, :], hT[:, :rows], w2sb[:],
                                         start=True, stop=True)
                        ht2t = wpool.tile([128, cfg.F2], DT.float32, tag="ht2t")
                        nc.vector.tensor_scalar_mul(ht2t[:rows, :], t2p[:rows, :],
                                                    dinv[:rows, t:t + 1])
                        nc.sync.dma_start(out=out_t[t * 128:t * 128 + rows, :],
                                          in_=ht2t[:rows, :])
                    else:
                        mx = wpool.tile([128, 1], DT.float32, tag="mx")
                        nc.vector.tensor_reduce(mx[:rows, :], z[:rows, :],
                                                axis=mybir.AxisListType.X,
                                                op=ALU.max)
                        zc = wpool.tile([128, cfg.F2], DT.float32, tag="zc")
                        nc.vector.tensor_scalar_sub(zc[:rows, :], z[:rows, :],
                                                    mx[:rows, :])
                        e = wpool.tile([128, cfg.F2], DT.float32, tag="e")
                        ssum = wpool.tile([128, 1], DT.float32, tag="ssum")
                        nc.scalar.activation(e[:rows, :], zc[:rows, :], ACTF.Exp,
                                             accum_out=ssum[:rows, :])
                        lse = wpool.tile([128, 1], DT.float32, tag="lse")
                        nc.scalar.activation(lse[:rows, :], ssum[:rows, :], ACTF.Ln)
                        o = wpool.tile([128, cfg.F2], DT.float32, tag="o")
                        nc.vector.tensor_scalar_sub(o[:rows, :], zc[:rows, :],
                                                    lse[:rows, :])
                        nc.sync.dma_start(out=out_t[t * 128:t * 128 + rows, :],
                                          in_=o[:rows, :])
    nc.compile()
    return nc


from contextlib import contextmanager


@contextmanager
def tile_pools(tc, cfg):
    with tc.tile_pool(name="const", bufs=1) as cpool, \
         tc.tile_pool(name="idx", bufs=1) as idxpool, \
         tc.tile_pool(name="g0", bufs=2) as g0, \
         tc.tile_pool(name="g1", bufs=2) as g1, \
         tc.tile_pool(name="g2", bufs=2) as g2, \
         tc.tile_pool(name="g3", bufs=2) as g3, \
         tc.tile_pool(name="S", bufs=4) as spool, \
         tc.tile_pool(name="work", bufs=3) as wpool, \
         tc.tile_pool(name="psum", bufs=2, space="PSUM") as ppool:
        yield cpool, idxpool, [g0, g1, g2, g3][:cfg.SEG], spool, wpool, ppool


# ----------------------------------------------------------------------------
# Runner
# ----------------------------------------------------------------------------

def _install_ntff_hook():
    try:
        import antenv
        if "antenv.axon_hooks" not in sys.modules:
            from trn_agent_boot.trn_boot import _ntff_profile_via_ctypes
            hooks = types.ModuleType("antenv.axon_hooks")
            holder = {"hook": _ntff_profile_via_ctypes("/opt/axon/libaxon_pjrt.so")}
            hooks.get_axon_ntff_profile_hook = lambda: holder["hook"]
            hooks.set_axon_ntff_profile_hook = lambda h: holder.__setitem__("hook", h)
            sys.modules["antenv.axon_hooks"] = hooks
            antenv.axon_hooks = hooks
    except Exception:
        pass


_CACHE = {}
LAST_EXEC_NS = []


def _get_programs(cfg, meta, key):
    if key not in _CACHE:
        _CACHE[key] = (build_transform1(cfg),
                       build_agg(cfg, meta, 1),
                       build_agg(cfg, meta, 2))
    return _CACHE[key]


def kernel(x, edge_index, W1, b1, W2, b2):
    cfg = Cfg()
    x = np.asarray(x, dtype=np.float32)
    edge_index = np.asarray(edge_index)
    W1 = np.asarray(W1, dtype=np.float32)
    b1 = np.asarray(b1, dtype=np.float32)
    W2 = np.asarray(W2, dtype=np.float32)
    b2 = np.asarray(b2, dtype=np.float32)

    trace = os.environ.get("GCN_TRACE", "0") == "1"
    if trace:
        _install_ntff_hook()

    meta = preprocess(cfg, edge_index)
    key = hash(edge_index.tobytes())
    p1, p2, p3 = _get_programs(cfg, meta, key)

    iota_v = np.tile(np.arange(128, dtype=np.float32), (128, 1))
    ident_v = np.eye(128, dtype=np.float32)
    b1b = np.broadcast_to(b1, (128, cfg.F1)).copy()
    b2b = np.broadcast_to(b2, (128, cfg.F2)).copy()
    cores = list(range(cfg.NCORES))

    global LAST_EXEC_NS
    LAST_EXEC_NS = []

    # Launch 1: transform
    maps1 = [{"x": x[c * cfg.NPC:(c + 1) * cfg.NPC], "w1": W1,
              "cnt": meta.cnt_dev[c], "ident": ident_v} for c in cores]
    r1 = run_bass_kernel_spmd(p1, maps1, cores, trace=trace)
    LAST_EXEC_NS.append(r1.exec_time_ns)
    ht1 = np.concatenate([r1.results[c]["ht1"] for c in cores], axis=0)

    # Launch 2: layer-1 aggregation + transform-2
    segs1 = {f"tab{si}": ht1[si * cfg.SEGSZ:(si + 1) * cfg.SEGSZ]
             for si in range(cfg.SEG)}
    maps2 = [{**segs1, "own": ht1[c * cfg.NPC:(c + 1) * cfg.NPC],
              "cnt": meta.cnt_dev[c], "idx": meta.idx_dev[c],
              "dl": meta.dl_dev[c], "iota": iota_v, "bvec": b1b,
              "ident": ident_v, "w2": W2} for c in cores]
    r2 = run_bass_kernel_spmd(p2, maps2, cores, trace=trace)
    LAST_EXEC_NS.append(r2.exec_time_ns)
    ht2 = np.concatenate([r2.results[c]["ht2"] for c in cores], axis=0)

    # Launch 3: layer-2 aggregation + log_softmax
    segs2 = {f"tab{si}": ht2[si * cfg.SEGSZ:(si + 1) * cfg.SEGSZ]
             for si in range(cfg.SEG)}
    maps3 = [{**segs2, "own": ht2[c * cfg.NPC:(c + 1) * cfg.NPC],
              "cnt": meta.cnt_dev[c], "idx": meta.idx_dev[c],
              "dl": meta.dl_dev[c], "iota": iota_v, "bvec": b2b}
             for c in cores]
    r3 = run_bass_kernel_spmd(p3, maps3, cores, trace=trace)
    LAST_EXEC_NS.append(r3.exec_time_ns)
    out = np.concatenate([r3.results[c]["out"] for c in cores], axis=0)
    return out


# revision 9
# speedup vs baseline: 2.3225x; 2.3225x over previous
"""Distributed 2-layer GCN (PyG GCNConv semantics) on 8 Trainium2 NeuronCores.

Strategy (per sharding hint): nodes are sharded across the 8 cores
(12500 nodes each); edges are bucketed by destination core/tile via 1D
graph partitioning on the host. Three SPMD launches:

  1. transform1:  ht1 = dinv * (x @ W1)          (each core: its node shard)
     -- host concatenates the 8 shards into the full ht1 table --
  2. agg1+xform2: per dst tile: one-hot matmul segmented sum over
     dma_gather'ed ht1[src] rows, + self loop + bias + relu, then
     ht2 = dinv * (h1 @ W2)
     -- host concatenates ht2 shards --
  3. agg2+logsoftmax: same aggregation over ht2, + bias, log_softmax.

Aggregation: edges sorted into per-(dst-tile, src-segment) buckets
padded to 128-edge blocks. Each block reduces via a matmul whose
stationary operand is a one-hot selection matrix (iota == dst_local),
accumulating into PSUM per dst tile. Gather tables are bf16 (256-byte
rows); the one-hot matrices for a whole tile group are built with a
single wide vector-engine is_equal using stride-0 broadcast APs.
Gathers run on all 4 SWDGE queues to parallelize Q7 descriptor
generation (~3.7 ns/edge vs 8.6 single-queue).
"""

import os
import sys
import types
from contextlib import contextmanager

for _p in ("/opt/trn_rl_repo", "/root/.axon_site/_ro/trn_rl_repo", "/root/.axon_site"):
    if os.path.isdir(_p) and _p not in sys.path:
        sys.path.insert(0, _p)

import numpy as np
import ml_dtypes

from concourse import bass, bacc, tile
from concourse.bass_utils import run_bass_kernel_spmd

mybir = bass.mybir
DT = bass.mybir.dt
ALU = mybir.AluOpType
ACTF = mybir.ActivationFunctionType
BF16 = ml_dtypes.bfloat16

# ----------------------------------------------------------------------------
# Configuration
# ----------------------------------------------------------------------------

class Cfg:
    def __init__(self, N=100000, E=1600000, F0=256, F1=128, F2=64,
                 NCORES=8, SEG=4, TG=4):
        self.N = N
        self.E = E
        self.F0 = F0
        self.F1 = F1
        self.F2 = F2
        self.NCORES = NCORES
        self.NPC = N // NCORES            # nodes per core
        self.NT = -(-self.NPC // 128)     # dst tiles per core
        self.LAST_ROWS = self.NPC - (self.NT - 1) * 128
        self.SEG = SEG                    # src segments (int16 gather indices)
        assert N % SEG == 0
        self.SEGSZ = N // SEG
        assert self.SEGSZ <= 32767
        self.TG = TG                      # dst tiles per gather group
        self.NG = -(-self.NT // TG)
        self.groups = [list(range(g * TG, min((g + 1) * TG, self.NT)))
                       for g in range(self.NG)]


class Meta:
    """Edge partitioning metadata; identical across cores (static program)."""
    pass


def preprocess(cfg, edge_index):
    """1D graph partitioning of the edge list. Pure integer index work."""
    src = np.asarray(edge_index[0], dtype=np.int64)
    dst = np.asarray(edge_index[1], dtype=np.int64)

    cnt = np.bincount(dst, minlength=cfg.N).astype(np.int64)

    core = dst // cfg.NPC
    within = dst % cfg.NPC
    tile_id = within // 128
    dloc = within % 128
    seg = src // cfg.SEGSZ
    sloc = src % cfg.SEGSZ

    # bucket id (core, tile, seg); sort edges by (bucket, sloc) for locality
    bucket = (core * cfg.NT + tile_id) * cfg.SEG + seg
    order = np.argsort(bucket * np.int64(cfg.SEGSZ) + sloc, kind="stable")
    sloc_sorted = sloc[order].astype(np.int16)
    dloc_sorted = dloc[order].astype(np.float32)

    nbuckets = cfg.NCORES * cfg.NT * cfg.SEG
    bc = np.bincount(bucket, minlength=nbuckets).reshape(cfg.NCORES, cfg.NT, cfg.SEG)
    bstart = np.zeros(nbuckets + 1, np.int64)
    np.cumsum(bc.reshape(-1), out=bstart[1:])

    # static per-(tile, seg) block counts = max over cores, ceil to blocks
    nblk = -(-bc.max(axis=0) // 128)          # [NT, SEG]

    m = Meta()
    m.nblk = nblk
    # slot layout: group -> seg -> tiles in group -> blocks
    m.ns = np.zeros((cfg.NG, cfg.SEG), np.int64)       # slots per (group, seg)
    m.goff = np.zeros((cfg.NG, cfg.SEG), np.int64)     # global slot offset
    m.lco = {}                                          # (g, s, t) -> local block col
    off = 0
    for g, tiles in enumerate(cfg.groups):
        for s in range(cfg.SEG):
            m.goff[g, s] = off
            lc = 0
            for t in tiles:
                m.lco[(g, s, t)] = lc
                lc += int(nblk[t, s])
            m.ns[g, s] = lc * 128
            off += lc * 128
    m.tot = off
    assert m.tot % 128 == 0

    # per-core slot arrays
    idx_all = np.zeros((cfg.NCORES, m.tot), np.int16)
    dl_all = np.full((cfg.NCORES, m.tot), -1.0, np.float32)
    for c in range(cfg.NCORES):
        for g, tiles in enumerate(cfg.groups):
            for s in range(cfg.SEG):
                for t in tiles:
                    b = (c * cfg.NT + t) * cfg.SEG + s
                    k = int(bc[c, t, s])
                    if k == 0:
                        continue
                    e0 = int(bstart[b])
                    o = int(m.goff[g, s]) + m.lco[(g, s, t)] * 128
                    idx_all[c, o:o + k] = sloc_sorted[e0:e0 + k]
                    dl_all[c, o:o + k] = dloc_sorted[e0:e0 + k]

    # device layouts
    # idx: slot j -> [j % 16, j // 16], replicated over the 8 stripes of 16
    idx_dev = np.ascontiguousarray(
        np.tile(idx_all.reshape(cfg.NCORES, m.tot // 16, 16).transpose(0, 2, 1),
                (1, 8, 1)))
    # dstloc: slot j -> [j % 128, j // 128]; small ints, exact in bf16
    dl_dev = np.ascontiguousarray(
        dl_all.reshape(cfg.NCORES, m.tot // 128, 128).transpose(0, 2, 1)
    ).astype(BF16)

    # degree counts per core as f32 [128, NT] (node t*128+p <-> [p, t])
    pad = cfg.NT * 128 - cfg.NPC
    cnt_dev = np.zeros((cfg.NCORES, 128, cfg.NT), np.float32)
    for c in range(cfg.NCORES):
        cc = cnt[c * cfg.NPC:(c + 1) * cfg.NPC]
        cc = np.concatenate([cc, np.zeros(pad, np.int64)])
        cnt_dev[c] = cc.reshape(cfg.NT, 128).T.astype(np.float32)

    m.idx_dev = idx_dev
    m.dl_dev = dl_dev
    m.cnt_dev = cnt_dev
    return m


# ----------------------------------------------------------------------------
# Program builders
# ----------------------------------------------------------------------------

def _dinv_tiles(nc, pool, cnt_in, cfg):
    """dinv = 1/sqrt(cnt + 1) as an SBUF [128, NT] f32 tile."""
    cnt_sb = pool.tile([128, cfg.NT], DT.float32, tag="cnt")
    nc.sync.dma_start(out=cnt_sb[:], in_=cnt_in[:])
    deg = pool.tile([128, cfg.NT], DT.float32, tag="deg")
    nc.vector.tensor_scalar_add(deg[:], cnt_sb[:], 1.0)
    sq = pool.tile([128, cfg.NT], DT.float32, tag="sq")
    nc.scalar.sqrt(sq[:], deg[:])
    dinv = pool.tile([128, cfg.NT], DT.float32, tag="dinv")
    nc.vector.reciprocal(dinv[:], sq[:])
    return dinv


def build_transform1(cfg):
    """ht1 = dinv * (x @ W1) for the local node shard (f32 + bf16 copies)."""
    nc = bacc.Bacc(None, target_bir_lowering=False)
    x_in = nc.declare_dram_parameter("x", [cfg.NPC, cfg.F0], DT.float32, isOutput=False)
    w1_in = nc.declare_dram_parameter("w1", [cfg.F0, cfg.F1], DT.float32, isOutput=False)
    cnt_in = nc.declare_dram_parameter("cnt", [128, cfg.NT], DT.float32, isOutput=False)
    id_in = nc.declare_dram_parameter("ident", [128, 128], DT.float32, isOutput=False)
    ht_out = nc.declare_dram_parameter("ht1", [cfg.NPC, cfg.F1], DT.float32, isOutput=True)
    htb_out = nc.declare_dram_parameter("ht1b", [cfg.NPC, cfg.F1], DT.bfloat16, isOutput=True)

    KB = cfg.F0 // 128
    with tile.TileContext(nc) as tc:
        with tc.tile_pool(name="const", bufs=1) as cpool, \
             tc.tile_pool(name="work", bufs=3) as wpool, \
             tc.tile_pool(name="psum", bufs=2, space="PSUM") as ppool:
            ident = cpool.tile([128, 128], DT.float32, tag="ident")
            nc.sync.dma_start(out=ident[:], in_=id_in[:])
            dinv = _dinv_tiles(nc, cpool, cnt_in, cfg)
            w1sb = []
            for kb in range(KB):
                w = cpool.tile([128, cfg.F1], DT.float32, tag=f"w1_{kb}")
                nc.sync.dma_start(out=w[:], in_=w1_in[kb * 128:(kb + 1) * 128, :])
                w1sb.append(w)

            for t in range(cfg.NT):
                rows = cfg.LAST_ROWS if t == cfg.NT - 1 else 128
                xt = wpool.tile([128, cfg.F0], DT.float32, tag="xt")
                nc.sync.dma_start(out=xt[:rows, :],
                                  in_=x_in[t * 128:t * 128 + rows, :])
                xTs = []
                for kb in range(KB):
                    xTp = ppool.tile([128, 128], DT.float32, tag="xTp")
                    nc.tensor.transpose(xTp[:, :rows],
                                        xt[:rows, kb * 128:(kb + 1) * 128],
                                        ident[:rows, :rows])
                    xTc = wpool.tile([128, 128], DT.float32, tag=f"xT_{kb}")
                    nc.vector.tensor_copy(xTc[:, :rows], xTp[:, :rows])
                    xTs.append(xTc)
                hp = ppool.tile([128, cfg.F1], DT.float32, tag="hp")
                for kb in range(KB):
                    nc.tensor.matmul(hp[:rows, :], xTs[kb][:, :rows], w1sb[kb][:],
                                     start=(kb == 0), stop=(kb == KB - 1))
                hts = wpool.tile([128, cfg.F1], DT.float32, tag="hts")
                nc.vector.tensor_scalar_mul(hts[:rows, :], hp[:rows, :],
                                            dinv[:rows, t:t + 1])
                nc.sync.dma_start(out=ht_out[t * 128:t * 128 + rows, :],
                                  in_=hts[:rows, :])
                htb = wpool.tile([128, cfg.F1], DT.bfloat16, tag="htb")
                nc.vector.tensor_copy(htb[:rows, :], hts[:rows, :])
                nc.sync.dma_start(out=htb_out[t * 128:t * 128 + rows, :],
                                  in_=htb[:rows, :])
    nc.compile()
    return nc


@contextmanager
def _agg_pools(tc, cfg):
    with tc.tile_pool(name="const", bufs=1) as cpool, \
         tc.tile_pool(name="idx", bufs=1) as idxpool, \
         tc.tile_pool(name="g0", bufs=2) as g0, \
         tc.tile_pool(name="g1", bufs=2) as g1, \
         tc.tile_pool(name="g2", bufs=2) as g2, \
         tc.tile_pool(name="g3", bufs=2) as g3, \
         tc.tile_pool(name="S", bufs=2) as spool, \
         tc.tile_pool(name="work", bufs=3) as wpool, \
         tc.tile_pool(name="psum", bufs=2, space="PSUM") as ppool:
        yield cpool, idxpool, [g0, g1, g2, g3][:cfg.SEG], spool, wpool, ppool


def build_agg(cfg, meta, layer):
    """layer 1: aggregate ht1 -> h1 -> ht2 = dinv*(h1 @ W2). Outputs ht2 (f32)
       and ht2b (bf16, rows padded to 128 elems so gathers stay 256B).
       layer 2: aggregate ht2 -> +b2 -> log_softmax. Output "out"."""
    FIN = cfg.F1 if layer == 1 else cfg.F2    # aggregated feature width
    FROW = cfg.F1 if layer == 1 else 128      # bf16 gather row width (256B)
    nc = bacc.Bacc(None, target_bir_lowering=False, num_swdge_queues=4)
    tab_in = [nc.declare_dram_parameter(f"tab{si}", [cfg.SEGSZ, FROW], DT.bfloat16,
                                        isOutput=False) for si in range(cfg.SEG)]
    own_in = nc.declare_dram_parameter("own", [cfg.NPC, FIN], DT.float32, isOutput=False)
    cnt_in = nc.declare_dram_parameter("cnt", [128, cfg.NT], DT.float32, isOutput=False)
    idx_in = nc.declare_dram_parameter("idx", [128, meta.tot // 16], DT.int16, isOutput=False)
    dl_in = nc.declare_dram_parameter("dl", [128, meta.tot // 128], DT.bfloat16, isOutput=False)
    iota_in = nc.declare_dram_parameter("iota", [128, 128], DT.bfloat16, isOutput=False)
    b_in = nc.declare_dram_parameter("bvec", [128, FIN], DT.float32, isOutput=False)
    if layer == 1:
        id_in = nc.declare_dram_parameter("ident", [128, 128], DT.float32, isOutput=False)
        w2_in = nc.declare_dram_parameter("w2", [cfg.F1, cfg.F2], DT.float32, isOutput=False)
        out_t = nc.declare_dram_parameter("ht2", [cfg.NPC, cfg.F2], DT.float32, isOutput=True)
        outb_t = nc.declare_dram_parameter("ht2b", [cfg.NPC, 128], DT.bfloat16, isOutput=True)
    else:
        out_t = nc.declare_dram_parameter("out", [cfg.NPC, cfg.F2], DT.float32, isOutput=True)

    nblk = meta.nblk
    with tile.TileContext(nc) as tc:
        with _agg_pools(tc, cfg) as (cpool, idxpool, gpools, spool, wpool, ppool):
            iota = cpool.tile([128, 128], DT.bfloat16, tag="iota")
            nc.sync.dma_start(out=iota[:], in_=iota_in[:])
            bvec = cpool.tile([128, FIN], DT.float32, tag="bvec")
            nc.sync.dma_start(out=bvec[:], in_=b_in[:])
            dinv = _dinv_tiles(nc, cpool, cnt_in, cfg)
            if layer == 1:
                ident = cpool.tile([128, 128], DT.float32, tag="ident")
                nc.sync.dma_start(out=ident[:], in_=id_in[:])
                w2sb = cpool.tile([128, cfg.F2], DT.float32, tag="w2")
                nc.sync.dma_start(out=w2sb[:], in_=w2_in[:])

            idx_sb = idxpool.tile([128, meta.tot // 16], DT.int16, tag="idx")
            nc.sync.dma_start(out=idx_sb[:], in_=idx_in[:])
            dl_sb = idxpool.tile([128, meta.tot // 128], DT.bfloat16, tag="dl")
            nc.sync.dma_start(out=dl_sb[:], in_=dl_in[:])

            for g, tiles in enumerate(cfg.groups):
                c0 = int(meta.goff[g, 0]) // 128
                nbg = sum(int(meta.ns[g, s]) for s in range(cfg.SEG)) // 128
                Gt = {}
                for s in range(cfg.SEG):
                    ns = int(meta.ns[g, s])
                    if ns == 0:
                        continue
                    Gs = gpools[s].tile([128, ns // 128, FROW], DT.bfloat16,
                                        tag=f"G{s}")
                    o16 = int(meta.goff[g, s]) // 16
                    nc.gpsimd.dma_gather(
                        out_ap=Gs[:],
                        in_ap=tab_in[s][:, :],
                        idxs_ap=idx_sb[:, o16:o16 + ns // 16],
                        num_idxs=ns,
                        num_idxs_reg=ns,
                        elem_size=FROW,
                        single_packet=False,
                        queue_num=s,
                    )
                    Gt[s] = Gs

                # one wide one-hot build for the whole group's block range
                Sw = spool.tile([128, nbg, 128], DT.bfloat16, tag="S")
                nc.vector.tensor_tensor(
                    Sw[:],
                    iota[:].unsqueeze(1).broadcast_to((128, nbg, 128)),
                    dl_sb[:, c0:c0 + nbg].unsqueeze(2).broadcast_to((128, nbg, 128)),
                    op=ALU.is_equal)

                for t in tiles:
                    rows = cfg.LAST_ROWS if t == cfg.NT - 1 else 128
                    nbt = int(nblk[t].sum())
                    acc = ppool.tile([128, FIN], DT.float32, tag="acc")
                    bi = 0
                    for s in range(cfg.SEG):
                        lco = meta.lco[(g, s, t)]
                        sc0 = int(meta.goff[g, s]) // 128 - c0
                        for j in range(int(nblk[t, s])):
                            nc.tensor.matmul(acc[:], Sw[:, sc0 + lco + j, :],
                                             Gt[s][:, lco + j, 0:FIN],
                                             start=(bi == 0), stop=(bi == nbt - 1))
                            bi += 1
                    assert bi == nbt and nbt > 0

                    ownt = wpool.tile([128, FIN], DT.float32, tag="own")
                    nc.sync.dma_start(out=ownt[:rows, :],
                                      in_=own_in[t * 128:t * 128 + rows, :])
                    z = wpool.tile([128, FIN], DT.float32, tag="z")
                    nc.vector.tensor_tensor(z[:rows, :], acc[:rows, :],
                                            ownt[:rows, :], op=ALU.add)
                    nc.vector.tensor_scalar_mul(z[:rows, :], z[:rows, :],
                                                dinv[:rows, t:t + 1])
                    nc.vector.tensor_tensor(z[:rows, :], z[:rows, :],
                                            bvec[:rows, :], op=ALU.add)
                    if layer == 1:
                        h1 = wpool.tile([128, cfg.F1], DT.float32, tag="h1")
                        nc.scalar.activation(h1[:rows, :], z[:rows, :], ACTF.Relu)
                        hTp = ppool.tile([128, 128], DT.float32, tag="hTp")
                        nc.tensor.transpose(hTp[:, :rows], h1[:rows, :],
                                            ident[:rows, :rows])
                        hT = wpool.tile([128, 128], DT.float32, tag="hT")
                        nc.vector.tensor_copy(hT[:, :rows], hTp[:, :rows])
                        t2p = ppool.tile([128, cfg.F2], DT.float32, tag="t2p")
                        nc.tensor.matmul(t2p[:rows, :], hT[:, :rows], w2sb[:],
                                         start=True, stop=True)
                        ht2t = wpool.tile([128, cfg.F2], DT.float32, tag="ht2t")
                        nc.vector.tensor_scalar_mul(ht2t[:rows, :], t2p[:rows, :],
                                                    dinv[:rows, t:t + 1])
                        nc.sync.dma_start(out=out_t[t * 128:t * 128 + rows, :],
                                          in_=ht2t[:rows, :])
                        htb = wpool.tile([128, 128], DT.bfloat16, tag="htb")
                        nc.vector.tensor_copy(htb[:rows, 0:cfg.F2], ht2t[:rows, :])
                        nc.vector.tensor_copy(htb[:rows, cfg.F2:2 * cfg.F2],
                                              ht2t[:rows, :])
                        nc.sync.dma_start(
                            out=outb_t[t * 128:t * 128 + rows, :],
                            in_=htb[:rows, :])
                    else:
                        mx = wpool.tile([128, 1], DT.float32, tag="mx")
                        nc.vector.tensor_reduce(mx[:rows, :], z[:rows, :],
                                                axis=mybir.AxisListType.X,
                                                op=ALU.max)
                        zc = wpool.tile([128, cfg.F2], DT.float32, tag="zc")
                        nc.vector.tensor_scalar_sub(zc[:rows, :], z[:rows, :],
                                                    mx[:rows, :])
                        e = wpool.tile([128, cfg.F2], DT.float32, tag="e")
                        ssum = wpool.tile([128, 1], DT.float32, tag="ssum")
                        nc.scalar.activation(e[:rows, :], zc[:rows, :], ACTF.Exp,
                                             accum_out=ssum[:rows, :])
                        lse = wpool.tile([128, 1], DT.float32, tag="lse")
                        nc.scalar.activation(lse[:rows, :], ssum[:rows, :], ACTF.Ln)
                        o = wpool.tile([128, cfg.F2], DT.float32, tag="o")
                        nc.vector.tensor_scalar_sub(o[:rows, :], zc[:rows, :],
                                                    lse[:rows, :])
                        nc.sync.dma_start(out=out_t[t * 128:t * 128 + rows, :],
                                          in_=o[:rows, :])
    nc.compile()
    return nc


# ----------------------------------------------------------------------------
# Runner
# ----------------------------------------------------------------------------

def _install_ntff_hook():
    try:
        import antenv
        if "antenv.axon_hooks" not in sys.modules:
            from trn_agent_boot.trn_boot import _ntff_profile_via_ctypes
            hooks = types.ModuleType("antenv.axon_hooks")
            holder = {"hook": _ntff_profile_via_ctypes("/opt/axon/libaxon_pjrt.so")}
            hooks.get_axon_ntff_profile_hook = lambda: holder["hook"]
            hooks.set_axon_ntff_profile_hook = lambda h: holder.__setitem__("hook", h)
            sys.modules["antenv.axon_hooks"] = hooks
            antenv.axon_hooks = hooks
    except Exception:
        pass


_CACHE = {}
LAST_EXEC_NS = []


def _get_programs(cfg, meta, key):
    if key not in _CACHE:
        _CACHE[key] = (build_transform1(cfg),
                       build_agg(cfg, meta, 1),
                       build_agg(cfg, meta, 2))
    return _CACHE[key]


def kernel(x, edge_index, W1, b1, W2, b2):
    cfg = Cfg()
    x = np.asarray(x, dtype=np.float32)
    edge_index = np.asarray(edge_index)
    W1 = np.asarray(W1, dtype=np.float32)
    b1 = np.asarray(b1, dtype=np.float32)
    W2 = np.asarray(W2, dtype=np.float32)
    b2 = np.asarray(b2, dtype=np.float32)

    trace = os.environ.get("GCN_TRACE", "0") == "1"
    if trace:
        _install_ntff_hook()

    meta = preprocess(cfg, edge_index)
    key = hash(edge_index.tobytes())
    p1, p2, p3 = _get_programs(cfg, meta, key)

    iota_v = np.tile(np.arange(128, dtype=np.float32), (128, 1)).astype(BF16)
    ident_v = np.eye(128, dtype=np.float32)
    b1b = np.broadcast_to(b1, (128, cfg.F1)).copy()
    b2b = np.broadcast_to(b2, (128, cfg.F2)).copy()
    cores = list(range(cfg.NCORES))

    global LAST_EXEC_NS
    LAST_EXEC_NS = []

    # Launch 1: transform
    maps1 = [{"x": x[c * cfg.NPC:(c + 1) * cfg.NPC], "w1": W1,
              "cnt": meta.cnt_dev[c], "ident": ident_v} for c in cores]
    r1 = run_bass_kernel_spmd(p1, maps1, cores, trace=trace)
    LAST_EXEC_NS.append(r1.exec_time_ns)
    ht1 = np.concatenate([r1.results[c]["ht1"] for c in cores], axis=0)
    ht1b = np.concatenate([r1.results[c]["ht1b"] for c in cores], axis=0)

    # Launch 2: layer-1 aggregation + transform-2
    segs1 = {f"tab{si}": ht1b[si * cfg.SEGSZ:(si + 1) * cfg.SEGSZ]
             for si in range(cfg.SEG)}
    maps2 = [{**segs1, "own": ht1[c * cfg.NPC:(c + 1) * cfg.NPC],
              "cnt": meta.cnt_dev[c], "idx": meta.idx_dev[c],
              "dl": meta.dl_dev[c], "iota": iota_v, "bvec": b1b,
              "ident": ident_v, "w2": W2} for c in cores]
    r2 = run_bass_kernel_spmd(p2, maps2, cores, trace=trace)
    LAST_EXEC_NS.append(r2.exec_time_ns)
    ht2 = np.concatenate([r2.results[c]["ht2"] for c in cores], axis=0)
    ht2b = np.concatenate([r2.results[c]["ht2b"] for c in cores], axis=0)

    # Launch 3: layer-2 aggregation + log_softmax
    segs2 = {f"tab{si}": ht2b[si * cfg.SEGSZ:(si + 1) * cfg.SEGSZ]
             for si in range(cfg.SEG)}
    maps3 = [{**segs2, "own": ht2[c * cfg.NPC:(c + 1) * cfg.NPC],
              "cnt": meta.cnt_dev[c], "idx": meta.idx_dev[c],
              "dl": meta.dl_dev[c], "iota": iota_v, "bvec": b2b}
             for c in cores]
    r3 = run_bass_kernel_spmd(p3, maps3, cores, trace=trace)
    LAST_EXEC_NS.append(r3.exec_time_ns)
    out = np.concatenate([r3.results[c]["out"] for c in cores], axis=0)
    return out
